# revision 42
# baseline (speedup 1.0000x reference)
"""CTRGC-style GNN message passing kernel for Trainium2 (8 NeuronCores).

Data-parallel over batch N=64: each of 8 cores processes S=8 samples.
Math per sample:
  xsum = sum_t x                          [C,V]
  x1m/x2m/q/k = (W/T) @ xsum + b          [R,V]   (folded scales)
  d    = tanh(x1m[u] - x2m[v])            [R,V,V] (stored (v,u) free-major)
  res  = tanh(softmax_v(5*q[u]*k[v]))
  adj  = W4 @ (d+res) + A + 2*b4          [O,V,V]
  x3   = W3 @ x                           [O,T,V] (b3 == 0)
  out  = einsum('ouv,otv->otu', adj, x3)  [O,T,U]

Design (f16 datapath with an fp8-e3m4 x3 restripe; ~90us/core vs 92us
for the all-f16 version, rel err ~1.3e-2 vs the 2e-2 gate):
 - matmul operands f16 except the restriped x3, which is drained to
   fp8 e3m4 carrying 4*x3 (the x4 centers its N(0,0.4) values in
   e3m4's normal range; folded via w3blk*4 and w4blk,arep/4 so
   out = (adj/4)@(4*x3)).  The fp8 slab halves the gather-side DMA:
   the (v,J,t) slab's read runs are 3.3KB (full 360GB/s) while the
   write runs are 256B and pay the <512B 2x penalty either way, so
   the restripe drops 36.7us -> 27.4us of DMA-engine time.
 - 2 samples packed per 128 partitions (block-diag weights); o-columns
   J = 13i+g hold o = 5g+i.
 - x3 restripe (o-part -> (i,v)-part) via TWO plain DMAs per sample
   through the DRAM slab; since J packs g contiguously the gather
   collapses to 3 AP dims.
 - adjT restriped through a (v, J, u) DRAM slab; the block-diagonal
   stationary is gathered straight from it with 5 diag-placement DMAs
   per sample into per-sample bd tiles whose off-diag zeros are
   memset once (engine copies can't do this: GPSIMD can't touch PSUM
   and compute partition bases must be 32-aligned; DMAs are exempt).
   The J=64 pad strips of both slabs are zero-scrubbed once so padded
   gathers read zeros; the g=12 agg matmul extends its lhsT free dim
   to 125 so the o=64 psum rows drain as zeros (host drops them).
 - aggregation: lhsT = block-diag adjT f16 [ (i,v), (i,u) ] stationary,
   rhs = fp8 img streamed 256 cols -> psum [(i,u), t]; two groups share
   one PSUM bank so one copy drains both; one contiguous out-DMA per
   sample; host reorders [s,(i,u),(g,t)] -> [s,o,t,u].
 - t-sum for the branch projections via a pairwise fp16 halving tree
   (DVE 2x mode; TensorReduce has no fast mode).
 - software pipeline per pair p: tree(p), x3(p), chain(p)+diag, img(p),
   agg(p-1); DMA queues chosen so no in-order SEQ blocks an independent
   later transfer (x/x3/img on SP, adj on Act, diag gathers + most
   outs on Pool/SWDGE whose SEQ is free).
"""

import numpy as np

S, C, T, V = 8, 64, 256, 25  # per-core samples and dims
O, R = 64, 8
NCORES = 8
NG = 13  # o-groups of 5 (last has 4)

JP = 65               # J padded so the (i=4,g=12) hole stays in-bounds
X3W = JP * T          # 16640 elems per v-row of the x3 slab
ADW = JP * V          # 1625 elems per v-row of the adj slab
IMW = NG * T          # 3328 img cols (g,t)
ADC = NG * V          # 325 dense adjT cols (g,u)
BDW = NG * 125        # 1625 block-diag cols

_cache = {}


def _o_of_j(j):
    # o-column order within a sample: J = 13i+g holds o = 5g+i
    if j < 52:
        i, g = j // 13, j % 13
    else:
        i, g = 4, j - 52
    return 5 * g + i


def _build_nc():
    import concourse.bass as bass
    import concourse.bacc as bacc
    import concourse.tile as tile
    import concourse.mybir as mybir

    f8 = mybir.dt.float8e3
    f16 = mybir.dt.float16
    f32 = mybir.dt.float32
    nc = bacc.Bacc("TRN2", target_bir_lowering=False, debug=False,
                   num_devices=NCORES)

    # x pre-transposed on host to [S, C, V, T] so the SBUF tile is
    # (v,t)-major with contiguous t runs (enables DVE 2x on the tree).
    x_d = nc.dram_tensor("x", [S, C, V, T], f16, kind="ExternalInput").ap()
    w3_d = nc.dram_tensor("w3blk", [128, 128], f16, kind="ExternalInput").ap()
    wb_d = nc.dram_tensor("wbblk", [128, 64], f16, kind="ExternalInput").ap()
    w4_d = nc.dram_tensor("w4blk", [16, 128], f16, kind="ExternalInput").ap()
    ar_d = nc.dram_tensor("arep", [128, 625], f16, kind="ExternalInput").ap()
    bb_d = nc.dram_tensor("bbvec", [16, 100], f32, kind="ExternalInput").ap()
    out_d = nc.dram_tensor("out", [S, 125, NG, T], f16,
                           kind="ExternalOutput").ap()
    scx = nc.dram_tensor("scx", [S, V, X3W], f8, kind="Internal").ap()
    sca = nc.dram_tensor("sca", [S, V, ADW], f16, kind="Internal").ap()

    TV = T * V  # 6400
    with tile.TileContext(nc) as tc:
        tc.race_detector_enabled = False
        from contextlib import ExitStack
        with ExitStack() as ctx:
            consts = ctx.enter_context(tc.tile_pool(name="consts", bufs=1))
            w3sb = consts.tile([128, 128], f16)
            wbsb = consts.tile([128, 64], f16)
            w4sb = consts.tile([16, 128], f16)
            arsb = consts.tile([128, 625], f16)
            bbsb = consts.tile([16, 100], f32)

            # per-sample block-diag adjT stationaries; off-diag zeros are
            # written once (only diag blocks are rewritten per sample).
            bd = [consts.tile([128, BDW], f16, name=f"bd{k}")
                  for k in range(S)]
            for k in range(S):
                nc.gpsimd.memset(bd[k][:], 0.0)

            # scrub the J=64 pad strip of both slabs once so the padded
            # gathers read defined memory (their targets are unused)
            z8 = consts.tile([25, 256], f8, name="z8")
            z16 = consts.tile([25, 25], f16, name="z16")
            nc.vector.memset(z8[:], 0.0)
            nc.vector.memset(z16[:], 0.0)
            nc.scalar.dma_start(
                bass.AP(scx.tensor, 64 * T,
                        [[X3W, V], [V * X3W, S], [1, T]]),
                z8[:].unsqueeze(1).broadcast_to([25, S, T]))
            nc.scalar.dma_start(
                bass.AP(sca.tensor, 64 * V,
                        [[ADW, V], [V * ADW, S], [1, V]]),
                z16[:].unsqueeze(1).broadcast_to([25, S, V]))

            import os as _os0
            _g = lambda k, d: int(_os0.environ.get(k, d))
            xpool = ctx.enter_context(tc.tile_pool(name="x", bufs=_g("K_XB", 3)))
            x3pool = ctx.enter_context(tc.tile_pool(name="x3", bufs=_g("K_X3B", 3)))
            ipool = ctx.enter_context(tc.tile_pool(name="img", bufs=_g("K_IB", 4)))
            opool = ctx.enter_context(tc.tile_pool(name="outsb", bufs=_g("K_OB", 5)))
            spool = ctx.enter_context(tc.tile_pool(name="small", bufs=_g("K_SB", 2)))
            tpool = ctx.enter_context(tc.tile_pool(name="tree", bufs=_g("K_TB", 1)))
            import os as _os
            _ppb = int(_os.environ.get("K_PPB", "3"))
            _pab = int(_os.environ.get("K_PAB", "2"))
            pp = ctx.enter_context(tc.tile_pool(name="ps", bufs=_ppb, space="PSUM"))
            pb = ctx.enter_context(tc.tile_pool(name="psb", bufs=1, space="PSUM"))
            pa = ctx.enter_context(tc.tile_pool(name="psa", bufs=_pab, space="PSUM"))

            # PSUM->SBUF copies: only Act/DVE may touch PSUM.
            # SBUF->SBUF (block-diag) copies may also use Pool (slow Q7).
            _rmap = {"a": nc.scalar, "v": nc.vector, "p": nc.gpsimd,
                     "s": nc.sync}
            rot = [_rmap[c] for c in _os.environ.get("K_ROT", "ava")]
            rk = [0]

            def copy(dst, src):
                e = rot[rk[0] % len(rot)]
                rk[0] += 1
                (e.copy if e is nc.scalar else e.tensor_copy)(dst, src)

            _DEF_SCHED = ("t0 x0 c0 t1 x1 i0 c1 i1 t2 c2 a0 x2 i2 "
                          "t3 c3 a1 x3 i3 a2 a3")

            # ---- load all x tiles up front (before the consts so the
            # first tree's input is in flight immediately) ----
            Xs = []
            xq = _os.environ.get("K_XQ", "ssss")
            for p in range(4):
                X = xpool.tile([128, TV], f16, tag="X")
                _rmap[xq[p % len(xq)]].dma_start(
                    X[:],
                    x_d[2 * p:2 * p + 2].rearrange("s c v t -> (s c) (v t)"))
                Xs.append(X)
            nc.scalar.dma_start(w3sb[:], w3_d)
            nc.scalar.dma_start(wbsb[:], wb_d)
            nc.scalar.dma_start(w4sb[:], w4_d)
            nc.scalar.dma_start(arsb[:], ar_d)
            nc.scalar.dma_start(bbsb[:], bb_d)

            imgs = {}

            def agg_block(q, only_s=None):
                # aggregation for pair q (deps long since satisfied)
                for s in ([only_s] if only_s is not None else range(2)):
                    n = 2 * q + s
                    img = imgs.pop((q, s))
                    osb = opool.tile([128, IMW], f16, tag="osb")
                    splitc = (_os.environ.get("K_SPLITC", "0") == "1"
                              or (q == 3 and s == 1 and
                                  _os.environ.get("K_SPLITL", "0") == "1"))
                    for gg in range(7):
                        g0 = 2 * gg
                        ncols = 512 if gg < 6 else 256
                        ag = pa.tile([128, 512], f32, tag="aggps")
                        for g in range(g0, min(g0 + 2, NG)):
                            kg = 125 if g < 12 else 100
                            # lhsT free is always 125: the g=12 pad
                            # cols are zeros (sca J=64 scrub), so the
                            # o=64 psum rows drain as zeros and the
                            # host drops them
                            nc.tensor.matmul(
                                ag[0:125, 256 * (g - g0):256 * (g - g0) + T],
                                bd[n][0:kg, 125 * g:125 * g + 125],
                                img[0:kg, T * g:T * g + T],
                                start=True, stop=True)
                        kgm = 125
                        if splitc and ncols == 512:
                            # halve drain latency: both PSUM-capable
                            # engines copy one group each in parallel
                            nc.scalar.copy(
                                osb[0:kgm, 512 * gg:512 * gg + 256],
                                ag[0:kgm, 0:256])
                            nc.vector.tensor_copy(
                                osb[0:kgm, 512 * gg + 256:512 * gg + 512],
                                ag[0:kgm, 256:512])
                        else:
                            copy(osb[0:kgm, 512 * gg:512 * gg + ncols],
                                 ag[0:kgm, 0:ncols])
                    oq = _rmap[_os.environ.get(
                        f"K_OQ{q}",
                        _os.environ.get("K_OQ", "s"))]
                    ofl = out_d[n].rearrange("p g t -> p (g t)")
                    if (q == 3 and s == 1 and
                            _os.environ.get("K_SPLITO", "1") == "1"):
                        # overlap the final out with its last agg copies
                        oq.dma_start(ofl[:, 0:1536], osb[0:125, 0:1536])
                        oq.dma_start(ofl[:, 1536:3072],
                                     osb[0:125, 1536:3072])
                        oq.dma_start(ofl[:, 3072:IMW],
                                     osb[0:125, 3072:IMW])
                    else:
                        oq.dma_start(ofl, osb[0:125, :])

            xsums = {}

            def tree_block(p):
                # pairwise f16 halving tree (DVE 2x mode; TensorReduce
                # has no fast mode) then a short reduce
                Xv = Xs[p][:].rearrange("p (v t) -> p v t", v=V)
                half = Xv
                for lvl, tk in enumerate([128, 64, 32, 16, 8]):
                    nxt = tpool.tile([128, V * tk], f16, tag=f"tr{lvl}")
                    nv = nxt[:].rearrange("p (v t) -> p v t", v=V)
                    nc.vector.tensor_add(nv, half[:, :, 0:tk],
                                         half[:, :, tk:2 * tk])
                    half = nv
                xsum = spool.tile([128, V], f16, tag=f"xsum{p % 2}")
                with nc.allow_low_precision("f16 t-sum; ~1e-3 rel err"):
                    nc.vector.tensor_reduce(xsum[:], half,
                                            axis=mybir.AxisListType.X,
                                            op=mybir.AluOpType.add)
                xsums[p] = xsum

            dq = [_rmap[c] for c in _os.environ.get("K_DQ", "ppppp")]

            def x3_block(p, mm_only=False, dma_only=False):
                # x3 = W3blk @ X (v-major free order), PSUM->SBUF fp16,
                # then the two-hop DRAM restripe into (i,v)-partitions.
                if dma_only:
                    return x3_dmas(p)
                Xv = Xs[p][:].rearrange("p (v t) -> p v t", v=V)
                x3sb = x3pool.tile([128, TV], f8, tag="x3sb")
                xr = _os.environ.get(f"K_XROT{p}", "av" if p == 2 else None)
                xrot = [_rmap[c] for c in xr] if xr else None
                for j in range(13):
                    w = 2 if j < 12 else 1
                    ps = pp.tile([128, 512 if j < 12 else 256], f32,
                                 tag="x3ps")
                    nc.tensor.matmul(ps[:], w3sb[:],
                                     Xv[:, 2 * j:2 * j + w, :],
                                     start=True, stop=True)
                    dst = x3sb[:, 512 * j:512 * j + 256 * w]
                    if xrot:
                        e = xrot[j % len(xrot)]
                        (e.copy if e is nc.scalar else e.tensor_copy)(
                            dst, ps[:])
                    else:
                        copy(dst, ps[:])
                x3sbs[p] = x3sb
                if not mm_only:
                    x3_dmas(p)

            x3sbs = {}

            def x3_dmas(p):
                x3sb = x3sbs[p]
                for s in range(2):
                    sb, n = s * 64, 2 * p + s
                    nc.sync.dma_start(
                        bass.AP(scx.tensor, n * V * X3W,
                                [[T, 64], [X3W, V], [1, T]]),
                        x3sb[sb:sb + 64, :].rearrange(
                            "j (v t) -> j v t", v=V))

            def img_block(p):
                for s in range(2):
                    n = 2 * p + s
                    img = ipool.tile([128, IMW], f8, tag=f"img{s}")
                    nc.sync.dma_start(
                        img[0:125, :],
                        bass.AP(scx.tensor, n * V * X3W,
                                [[NG * T, 5], [X3W, V], [1, IMW]]))
                    imgs[(p, s)] = img

            def chain_block(p):
                # branch projections -> [16, 100] = (x1m,x2m,q,k) x 25.
                # Branch biases are zero for this problem (asserted in
                # kernel()), so no bias add is needed.
                bps = pb.tile([16, 100], f32, tag="bps")
                for b in range(4):
                    nc.tensor.matmul(bps[:, 25 * b:25 * b + 25],
                                     wbsb[:, 16 * b:16 * b + 16],
                                     xsums[p][:], start=True, stop=True)
                bsb = spool.tile([16, 100], f32, tag="bsb")
                nc.scalar.copy(bsb[:], bps[:])
                x1 = bsb[:, 0:25].unsqueeze(1).broadcast_to([16, V, V])
                x2 = bsb[:, 25:50].unsqueeze(2).broadcast_to([16, V, V])
                qq = bsb[:, 50:75].unsqueeze(1).broadcast_to([16, V, V])
                kk = bsb[:, 75:100].unsqueeze(2).broadcast_to([16, V, V])
                dd = spool.tile([16, 625], f32, tag="dd")
                nc.vector.tensor_tensor(
                    dd[:].rearrange("p (v u) -> p v u", v=V), x1, x2,
                    op=mybir.AluOpType.subtract)
                dt_ = spool.tile([16, 625], f16, tag="dt")
                nc.scalar.activation(dt_[:], dd[:],
                                     mybir.ActivationFunctionType.Tanh)
                at = spool.tile([16, 625], f32, tag="at")
                nc.vector.tensor_tensor(
                    at[:].rearrange("p (v u) -> p v u", v=V), qq, kk,
                    op=mybir.AluOpType.mult)
                ea = spool.tile([16, 625], f32, tag="ea")
                nc.scalar.activation(ea[:], at[:],
                                     mybir.ActivationFunctionType.Exp)
                den = spool.tile([16, V], f32, tag="den")
                nc.vector.tensor_reduce(
                    den[:], ea[:].rearrange("p (v u) -> p u v", v=V),
                    axis=mybir.AxisListType.X, op=mybir.AluOpType.add)
                rden = spool.tile([16, V], f32, tag="rden")
                nc.vector.reciprocal(rden[:], den[:])
                sm = spool.tile([16, 625], f32, tag="sm")
                nc.vector.tensor_tensor(
                    sm[:].rearrange("p (v u) -> p v u", v=V),
                    ea[:].rearrange("p (v u) -> p v u", v=V),
                    rden[:].unsqueeze(1).broadcast_to([16, V, V]),
                    op=mybir.AluOpType.mult)
                res = spool.tile([16, 625], f16, tag="res")
                nc.scalar.activation(res[:], sm[:],
                                     mybir.ActivationFunctionType.Tanh)
                st = spool.tile([16, 625], f16, tag="st")
                nc.vector.tensor_add(st[:], dt_[:], res[:])

                aps_ = pb.tile([128, 625], f32, tag="adjps")
                nc.tensor.matmul(aps_[:, 0:512], w4sb[:], st[:, 0:512],
                                 start=True, stop=True)
                nc.tensor.matmul(aps_[:, 512:625], w4sb[:], st[:, 512:625],
                                 start=True, stop=True)
                adjsb = spool.tile([128, 625], f16, tag="adjsb")
                nc.vector.tensor_add(adjsb[:], aps_[:], arsb[:])

                for s in range(2):
                    sb, n = s * 64, 2 * p + s
                    if _os.environ.get("K_AJL", "vju") == "jvu":
                        nc.scalar.dma_start(
                            bass.AP(sca.tensor, n * V * ADW,
                                    [[625, 64], [1, 625]]),
                            adjsb[sb:sb + 64, :])
                    else:
                        nc.scalar.dma_start(
                            bass.AP(sca.tensor, n * V * ADW,
                                    [[V, 64], [ADW, V], [1, V]]),
                            adjsb[sb:sb + 64, :].rearrange(
                                "j (v u) -> j v u", v=V))
                if "d" not in _os.environ.get("K_SCHED", _DEF_SCHED):
                    diag_block(p)

            def diag_block(p, qs=None):
                dqp = ([_rmap[c] for c in qs] if qs
                       else ([_rmap[c] for c in
                              _os.environ.get("K_DQ3", "ppppp")]
                             if p == 3 else dq))
                for s in range(2):
                    n = 2 * p + s
                    # gather diag blocks straight into the stationary:
                    # bd[25i+v, 125g+25i+u] <- sca[n, 13i+g, 25v+u]
                    bd3 = bd[n][:].rearrange("p (g c) -> p g c", c=125)
                    for i in range(5):
                        dqp[i % len(dqp)].dma_start(
                            bd3[25 * i:25 * i + 25, :, 25 * i:25 * i + 25],
                            bass.AP(sca.tensor, n * V * ADW +
                                    (NG * i * 625
                                     if _os.environ.get("K_AJL", "vju")
                                     == "jvu" else NG * V * i),
                                    [[25, V], [625, NG], [1, V]]
                                    if _os.environ.get("K_AJL", "vju")
                                    == "jvu" else
                                    [[ADW, V], [V, NG], [1, V]]))

            # software pipeline over the blocks; dN=diag on default
            # queues, DN/EN = diag on sync/scalar HWDGE queues
            sched = _os.environ.get("K_SCHED", _DEF_SCHED)

            def zmem_block(k):
                nc.vector.memset(bd[4 + k][:], 0.0)

            blk = {"t": tree_block, "x": x3_block, "c": chain_block,
                   "a": agg_block, "i": img_block, "z": zmem_block,
                   "y": lambda p: x3_block(p, mm_only=True),
                   "h": lambda p: x3_block(p, dma_only=True),
                   "A": lambda q: agg_block(q, only_s=0),
                   "B": lambda q: agg_block(q, only_s=1),
                   "d": diag_block,
                   "D": lambda q: diag_block(q, "sssss"),
                   "E": lambda q: diag_block(q, "aaaaa")}
            for tok in sched.split():
                blk[tok[0]](int(tok[1]))
    nc.compile()
    return nc


def _get_nc():
    if "nc" not in _cache:
        _cache["nc"] = _build_nc()
    return _cache["nc"]


def _host_weights(A, W1, b1, W2, b2, W3, b3, W4, b4, W5, b5, w6, b6, w7, b7):
    f = np.float32
    s5 = np.sqrt(np.float32(5.0))
    Wq = (s5 * w6 * W5).astype(f)
    Wk = (s5 * w7 * W5).astype(f)
    bq = (s5 * (w6 * b5 + b6)).astype(f)  # [R]
    bk = (s5 * (w7 * b5 + b7)).astype(f)

    perm = np.array([_o_of_j(j) for j in range(64)])  # J -> o

    # x3 is carried x4-scaled in the e3m4 slab (centers its values in
    # e3m4's normal range); the adj path is scaled by 1/4 to fold the
    # factor back out: out = (adj/4) @ (4*x3)
    w3blk = np.zeros((128, 128), f)
    for s in range(2):
        w3blk[s * 64:(s + 1) * 64, s * 64:(s + 1) * 64] = 4.0 * W3[perm].T

    wbblk = np.zeros((128, 64), f)
    Wset = [W1, W2, Wq, Wk]
    for blk in range(4):
        for s in range(2):
            wbblk[s * 64:(s + 1) * 64, 16 * blk + s * 8: 16 * blk + s * 8 + 8] = \
                (Wset[blk] / T).T
    bbvec = np.zeros((16, 100), f)
    bset = [b1, b2, bq, bk]
    for blk in range(4):
        for s in range(2):
            bbvec[s * 8:(s + 1) * 8, 25 * blk:25 * blk + 25] = \
                bset[blk][:, None]

    w4blk = np.zeros((16, 128), f)
    for s in range(2):
        w4blk[s * 8:(s + 1) * 8, s * 64:(s + 1) * 64] = W4[perm].T / 4.0

    arep = np.zeros((128, 625), f)
    avu = (A.T).reshape(-1)  # index v*25+u -> A[u,v]
    for s in range(2):
        for j in range(64):
            arep[s * 64 + j, :] = (avu + 2.0 * b4[perm[j]]) / 4.0
    return (w3blk.astype(np.float16), wbblk.astype(np.float16),
            w4blk.astype(np.float16), arep.astype(np.float16), bbvec)


LAST_PROFILE = {}


def _in_maps(inputs):
    x = np.asarray(inputs["x"], np.float32)
    args = {k: np.asarray(np.float32(inputs[k]))
            for k in ["A", "W1", "b1", "W2", "b2", "W3", "b3", "W4", "b4",
                      "W5", "b5", "w6", "b6", "w7", "b7"]}
    w3blk, wbblk, w4blk, arep, bbvec = _host_weights(**args)
    x16 = np.ascontiguousarray(x.astype(np.float16).transpose(0, 1, 3, 2))
    in_maps = []
    for core in range(NCORES):
        in_maps.append({
            "x": np.ascontiguousarray(x16[core * S:(core + 1) * S]),
            "w3blk": w3blk, "wbblk": wbblk, "w4blk": w4blk,
            "arep": arep, "bbvec": bbvec,
        })
    return in_maps, args, bbvec


def bench(inputs, iters=30):
    """Wall-clock per-dispatch time of the compiled NEFF on the 8 cores.

    Builds the PJRT executable once (same lowering as
    run_bass_kernel_spmd under axon, minus output-buffer donation) and
    times repeated dispatches with on-device inputs. Returns
    (min_ns, median_ns) per dispatch — includes RPC/dispatch overhead,
    so it upper-bounds the HW exec time.
    """
    import time
    import jax
    import concourse.mybir as mybir
    from concourse import bass2jax
    from jax.experimental.shard_map import shard_map
    from jax.sharding import Mesh, PartitionSpec, NamedSharding

    x = np.asarray(inputs["x"], np.float32)
    args = {k: np.asarray(np.float32(inputs[k]))
            for k in ["A", "W1", "b1", "W2", "b2", "W3", "b3", "W4", "b4",
                      "W5", "b5", "w6", "b6", "w7", "b7"]}
    w3blk, wbblk, w4blk, arep, bbvec = _host_weights(**args)
    x16 = np.ascontiguousarray(x.astype(np.float16).transpose(0, 1, 3, 2))
    in_maps = []
    for core in range(NCORES):
        in_maps.append({
            "x": np.ascontiguousarray(x16[core * S:(core + 1) * S]),
            "w3blk": w3blk, "wbblk": wbblk, "w4blk": w4blk,
            "arep": arep, "bbvec": bbvec,
        })

    nc = _get_nc()
    bass2jax.install_neuronx_cc_hook()
    partition_name = (nc.partition_id_tensor.name
                      if nc.partition_id_tensor else None)
    in_names, out_names, out_avals, zero_outs = [], [], [], []
    for alloc in nc.m.functions[0].allocations:
        if not isinstance(alloc, mybir.MemoryLocationSet):
            continue
        name = alloc.memorylocations[0].name
        if alloc.kind == "ExternalInput":
            if name != partition_name:
                in_names.append(name)
        elif alloc.kind == "ExternalOutput":
            out_names.append(name)
            shape = tuple(alloc.tensor_shape)
            dtype = mybir.dt.np(alloc.dtype)
            out_avals.append(jax.core.ShapedArray(shape, dtype))
            zero_outs.append(np.zeros(shape, dtype))
    n_params = len(in_names)
    in_names.extend(out_names)
    if partition_name is not None:
        in_names.append(partition_name)

    def _body(*bargs):
        operands = list(bargs)
        if partition_name is not None:
            operands.append(bass2jax.partition_id_tensor())
        outs = bass2jax._bass_exec_p.bind(
            *operands,
            out_avals=tuple(out_avals),
            in_names=tuple(in_names),
            out_names=tuple(out_names),
            lowering_input_output_aliases=(),
            sim_require_finite=True,
            sim_require_nnan=True,
            nc=nc,
        )
        return tuple(outs)

    devices = jax.devices()[:NCORES]
    mesh = Mesh(np.asarray(devices), ("core",))
    n_outs = len(out_avals)
    in_specs = (PartitionSpec("core"),) * (n_params + n_outs)
    out_specs = (PartitionSpec("core"),) * n_outs
    fn = jax.jit(shard_map(_body, mesh=mesh, in_specs=in_specs,
                           out_specs=out_specs, check_rep=False),
                 keep_unused=True)
    per_core = [[np.asarray(m[name]) for name in in_names[:n_params]]
                for m in in_maps]
    concat = [np.concatenate([per_core[c][i] for c in range(NCORES)], axis=0)
              for i in range(n_params)]
    concat += [np.zeros((NCORES * z.shape[0], *z.shape[1:]), z.dtype)
               for z in zero_outs]
    sh = NamedSharding(mesh, PartitionSpec("core"))
    dev_args = [jax.device_put(a, sh) for a in concat]
    out = fn(*dev_args)
    jax.block_until_ready(out)
    times = []
    for _ in range(iters):
        t0 = time.perf_counter()
        out = fn(*dev_args)
        jax.block_until_ready(out)
        times.append(time.perf_counter() - t0)
    times.sort()
    return int(times[0] * 1e9), int(times[len(times) // 2] * 1e9)


def kernel(**inputs):
    import os
    import jax  # noqa: F401  (ensures axon/jax devices initialized)
    from concourse.bass_utils import run_bass_kernel_spmd

    in_maps, args, bbvec = _in_maps(inputs)
    nc = _get_nc()
    trace = bool(os.environ.get("KERNEL_TRACE"))
    tdir = os.environ.get("KERNEL_TRACE_DIR") or None
    res = run_bass_kernel_spmd(nc, in_maps, list(range(NCORES)),
                               trace=trace, tmpdir=tdir)
    LAST_PROFILE.update(
        exec_time_ns=res.exec_time_ns,
        mean_exec_time_ns=getattr(res, "mean_exec_time_ns", None),
        profile_json=res.profile_json,
        trace_dir=tdir)
    outs = []
    for core in range(NCORES):
        o = np.asarray(res.results[core]["out"])  # [S, 125, 13, 256] f16
        o = o.astype(np.float32).reshape(S, 5, 25, NG, T)
        o = o.transpose(0, 3, 1, 4, 2).reshape(S, 65, T, V)[:, :64]
        outs.append(o)
    full = np.concatenate(outs, axis=0)

    # The kernel folds all biases away; they are zero in this problem's
    # setup — assert rather than silently drop them.
    assert not np.any(args["b3"]), "kernel assumes b3 == 0"
    assert not np.any(bbvec), "kernel assumes zero branch biases"
    return np.ascontiguousarray(full.astype(np.float32))

